# revision 45
# baseline (speedup 1.0000x reference)
"""Trainium2 Bass kernel for nn_Block_38285338477091 (dense transformer block).

Strategy:
- Data-parallel over batch: 8 NeuronCores, one batch element [1024,1024] each,
  weights replicated, zero collectives.
- LoRA folded into the base weights host-side (x@W + ((x@A)@B)*s == x@(W + s*A@B)).
- Weight-stationary transposed-activation layout for linear layers;
  LayerNorm token-major (bn_stats over free dim), PE transpose to
  feature-major with the g/b affine fused into PSUM eviction.
- LN feature-transposes go through the DMA XBAR (dma_start_transpose, 8
  per LN phase) when the LN affine is trivial (g==1/b==0, guaranteed by
  the input spec) — no PE transposes or DVE evictions; a PE-transpose
  fallback handles general affines.
- Startup pipelining: v-GEMM groups (2 token-tiles each) are emitted
  interleaved with the per-tile LN chain so the PE starts real
  contraction work while btc tiles are still streaming in from HBM.
- Causal attention: heads packed two-per-array; softmax without
  max-subtraction; per-column sums ride the PV matmul as a 65th "ones"
  column of V; masked blocks skipped (k>q) or masked post-exp.
- Attention software pipeline: head-pair p+1's q/k GEMM is split into 4
  one-PSUM-bank groups (8 matmuls + immediate eviction) interleaved as
  PE fillers into p's exp-paced score/PV rounds, sized so pv(4) +
  qk-filler(1-2) + score psums fit the 8 PSUM banks.
- h ([4096,1024] bf16) stays fully in SBUF (no DRAM spill); attention-phase
  pools are scope-released before the MLP-phase pools allocate.
- m2 computed feature-major: out^T[c,t] accumulated 32-deep in PSUM, with
  the residual btc^T seeded into PSUM via PE transposes (no DVE adds),
  and the finished PSUM tile DMA'd straight to DRAM (out^T layout;
  host transposes back and adds m2_b).
- All weight DMAs are fully contiguous per partition (host pre-arranged
  [p, chunk, ...] layouts).
- All matmuls bf16 (fp32 accumulation in PSUM); LN statistics and softmax
  normalization in fp32.
"""
import numpy as np
import ml_dtypes
from contextlib import ExitStack

import concourse.bass as bass
import concourse.tile as tile
from concourse import bacc, mybir
from concourse.bass_utils import run_bass_kernel_spmd  # noqa: F401 (fallback path)

f32 = mybir.dt.float32
bf16 = mybir.dt.bfloat16
FT = mybir.ActivationFunctionType
OP = mybir.AluOpType

P = 128
T = 1024
C = 1024
EPS = 1e-5
N_CORES = 8

_NC_CACHE = {}


def build_nc(reps=1, trivial_affine=True):
    key = ("nc", reps, trivial_affine)
    if key in _NC_CACHE:
        return _NC_CACHE[key]
    nc = bacc.Bacc("TRN2", target_bir_lowering=False, debug=False)

    d_btc = nc.dram_tensor("btc", [T, C], f32, kind="ExternalInput").ap()
    d_wqk = nc.dram_tensor("wqk", [P, 16, 8, P], bf16, kind="ExternalInput").ap()
    d_wv = nc.dram_tensor("wv", [P, 8, C], bf16, kind="ExternalInput").ap()
    d_wpj = nc.dram_tensor("wpj", [P, 2, 8, 512], bf16, kind="ExternalInput").ap()
    d_wm1 = nc.dram_tensor("wm1", [P, 32, 8, P], bf16, kind="ExternalInput").ap()
    d_wm2 = nc.dram_tensor("wm2", [P, 8, 32, P], bf16, kind="ExternalInput").ap()
    # packed constants: [g1|b1|g2|b2 (8 each) | m1b (32) | identf (128)] f32,
    # [identb | dmask] bf16 — two DMAs total at kernel start.
    d_cf = nc.dram_tensor("constsf", [P, 192], f32, kind="ExternalInput").ap()
    d_cb = nc.dram_tensor("constsb", [P, 256], bf16, kind="ExternalInput").ap()

    d_outT = nc.dram_tensor("outT", [C, T], f32, kind="ExternalOutput").ap()

    with tile.TileContext(nc) as tc, ExitStack() as ctx:
        consts = ctx.enter_context(tc.tile_pool(name="consts", bufs=2))
        btcp = ctx.enter_context(tc.tile_pool(name="btcp", bufs=1))
        xnp = ctx.enter_context(tc.tile_pool(name="xnp", bufs=4))
        lnTp = ctx.enter_context(tc.tile_pool(name="lnTp", bufs=1))
        wq = ctx.enter_context(tc.tile_pool(name="wq", bufs=6))
        stp = ctx.enter_context(tc.tile_pool(name="stp", bufs=8))
        rp = ctx.enter_context(tc.tile_pool(name="rp", bufs=4))
        pp = ctx.enter_context(tc.tile_pool(name="pp", bufs=8, space="PSUM"))
        for _rep in range(reps):

          # ---- constants (2 packed DMAs; emitted after btc0 below) ----
          cf = consts.tile([P, 192], f32, tag="cf")
          cb = consts.tile([P, 256], bf16, tag="cb")
          identf = cf[:, 64:192]
          identb = cb[:, 0:P]
          dmask = cb[:, P:256]
          epst = consts.tile([P, 1], f32, tag="epst")
          nc.vector.memset(epst[:], EPS)
          # activation-table warmup: pull the ACT table load for Sqrt into
          # the initial DMA window instead of the first LN use.
          warm = consts.tile([P, 1], f32, tag="warm")
          nc.scalar.activation(warm[:], epst[:], FT.Sqrt, bias=epst[:])

          # ---- helpers ----
          def layernorm_tile(x_tile):
              bn6 = stp.tile([P, 2, 6], f32, tag="bn6")
              nc.vector.bn_stats(bn6[:, 0, :], x_tile[:, 0:512])
              nc.vector.bn_stats(bn6[:, 1, :], x_tile[:, 512:1024])
              mv = stp.tile([P, 2], f32, tag="mv")
              nc.vector.bn_aggr(mv[:], bn6[:])
              sq = stp.tile([P, 1], f32, tag="sq")
              nc.scalar.activation(sq[:], mv[:, 1:2], FT.Sqrt, bias=epst[:])
              rstd = stp.tile([P, 1], f32, tag="rstd")
              nc.vector.reciprocal(rstd[:], sq[:])
              xn = xnp.tile([P, T], bf16, tag="xn")
              nc.gpsimd.tensor_scalar(out=xn[:], in0=x_tile[:],
                                      scalar1=mv[:, 0:1], scalar2=rstd[:],
                                      op0=OP.subtract, op1=OP.mult)
              return xn

          def transposes_for(i, xn, dst_tiles, g0, b0):
              """dst[j][:, 128i:+128] = xn[:, 128j:+128].T with g/b affine.
              g0/b0 are column bases into the packed cf constants tile."""
              for j in range(8):
                  trp = pp.tile([P, P], bf16, tag="ps", name=f"tr{i}_{j}")
                  nc.tensor.transpose(trp[:], xn[:, P * j:P * (j + 1)], identb)
                  nc.vector.tensor_scalar(
                      out=dst_tiles[:, j, P * i:P * (i + 1)], in0=trp[:],
                      scalar1=cf[:, g0 + j:g0 + j + 1],
                      scalar2=cf[:, b0 + j:b0 + j + 1],
                      op0=OP.mult, op1=OP.add)

          with tc.tile_pool(name="attn", bufs=1) as ap_, \
               tc.tile_pool(name="qkp", bufs=2) as qkp, \
               tc.tile_pool(name="esp", bufs=12) as esp:
            # ---- phase A: btc + wv DMAs interleaved; LN1 + transposes + v ----
            wvch = ap_.tile([P, 8, C], bf16, tag="wv")
            btc_tiles = []

            xlnT3 = lnTp.tile([P, 8, T], bf16, tag="xT", name="xlnT3")
            v_tiles = []
            for tt in range(8):
                vt = ap_.tile([P, 1040], bf16, tag=f"v{tt}")
                vt3 = vt[:].rearrange("p (h d) -> p h d", d=65)
                nc.vector.memset(vt3[:, :, 64:65], 1.0)
                v_tiles.append(vt)

            xns = {}

            def emit_xbar_v(j):
                """transpose tile j to feature-major + its v-GEMM."""
                if trivial_affine:
                    nc.sync.dma_start_transpose(
                        xlnT3[:, :, P * j:P * (j + 1)], xns[j][:])
                else:
                    transposes_for(j, xns[j], xlnT3, 0, 8)
                pss = {cb: pp.tile([P, 512], f32, tag="ps",
                                   name=f"vps{j}_{cb}") for cb in range(2)}
                for kb in range(8):
                    for cb in range(2):
                        nc.tensor.matmul(
                            pss[cb][:],
                            xlnT3[:, kb, P * j:P * (j + 1)],
                            wvch[:, kb, 512 * cb:512 * (cb + 1)],
                            start=(kb == 0), stop=(kb == 7))
                vt3 = v_tiles[j][:].rearrange("p (h d) -> p h d", d=65)
                for cb in range(2):
                    nc.scalar.copy(
                        vt3[:, 8 * cb:8 * (cb + 1), 0:64],
                        pss[cb][:].rearrange("p (h d) -> p h d", d=64))

            # btc DMA + LN chain, with the XBAR transpose (and its v-GEMM)
            # lagged 2 tiles so its input is ready when the SP queue reaches
            # it — a dep-blocked DMA issue stalls every later DMA issue.
            for i in range(8):
                bt = btcp.tile([P, C], f32, tag=f"bt{i}")
                nc.sync.dma_start(bt[:], d_btc[P * i:P * (i + 1), :])
                btc_tiles.append(bt)
                if i == 0:
                    nc.sync.dma_start(cb[:], d_cb[:])
                    nc.sync.dma_start(cf[:], d_cf[:])
                    nc.sync.dma_start(wvch[:, 0:4, :], d_wv[:, 0:4, :])
                elif i == 1:
                    nc.sync.dma_start(wvch[:, 4:8, :], d_wv[:, 4:8, :])
                xns[i] = layernorm_tile(bt)
                if i >= 2:
                    emit_xbar_v(i - 2)
            for j in (6, 7):
                emit_xbar_v(j)

            # ---- phase B/C: q,k per head-pair + attention ----
            attout = [ap_.tile([P, T], bf16, tag=f"ao{p}", name=f"attout{p}")
                      for p in range(8)]

            def qk_group_ops(p):
                """q/k GEMM for head-pair p as 4 one-bank groups (8 mms +
                immediate evict each) so they can be interleaved as PE
                fillers into the previous p's exp-paced attention rounds
                without exhausting PSUM slots."""
                dsts, groups = {}, []
                for which, co in (("q", p), ("k", 8 + p)):
                    dst = qkp.tile([P, T], bf16, tag=which, name=f"{which}T{p}")
                    wch = wq.tile([P, 8, P], bf16, tag="wqk")
                    nc.sync.dma_start(wch[:], d_wqk[:, co, :, :])
                    dsts[which] = dst
                    for qb in range(2):
                        def g(wch=wch, dst=dst, qb=qb, which=which):
                            ps = pp.tile([P, 512], f32, tag="ps",
                                         name=f"qk{p}{which}{qb}")
                            for kb in range(8):
                                nc.tensor.matmul(
                                    ps[:], wch[:, kb, :],
                                    xlnT3[:, kb, 512 * qb:512 * (qb + 1)],
                                    start=(kb == 0), stop=(kb == 7))
                            nc.vector.tensor_copy(
                                dst[:, 512 * qb:512 * (qb + 1)], ps[:])
                        groups.append(g)
                return dsts, groups

            prch = {}
            proj_state = {}

            def proj_fill_groups():
                """proj(tt=0) partial matmuls as p=7's round fillers:
                attout[0..6] are complete by then; kb=7 + eviction happen
                in phase D. PSUM: pv(4) + proj(2) + s_ps(2) = 8 banks."""
                def mk(kbs):
                    def g():
                        if "pss" not in proj_state:
                            proj_state["pss"] = [
                                pp.tile([P, 512], f32, tag="ps",
                                        name=f"prps0_{cb}") for cb in range(2)]
                        pss = proj_state["pss"]
                        for kb in kbs:
                            for cb in range(2):
                                nc.tensor.matmul(
                                    pss[cb][:], attout[kb][:, 0:P],
                                    prch[cb][:, kb, :],
                                    start=(kb == 0), stop=False)
                    return g
                return [mk([0, 1]), mk([2, 3]), mk([4, 5]), mk([6])]

            qk_cur, groups = qk_group_ops(0)
            for g in groups:
                g()
            for p in range(8):
                if p == 6:
                    for cb in range(2):
                        w = ap_.tile([P, 8, 512], bf16, tag=f"pj{cb}")
                        nc.sync.dma_start(w[:], d_wpj[:, cb, :, :])
                        prch[cb] = w
                dsts_nxt, groups_nxt = (qk_group_ops(p + 1) if p < 7
                                        else (None, proj_fill_groups()))
                qT, kT = qk_cur["q"], qk_cur["k"]
                pv_ps = {(qb, u): pp.tile([65, 512], f32, tag="ps",
                                          name=f"pv{p}_{qb}_{u}")
                         for qb in range(2) for u in range(2)}

                def emit_exp(es, s_ps, mi):
                    if 0 <= mi < 4:
                        d0 = 128 * mi
                        nc.scalar.activation(es[:, d0:512], s_ps[:, d0:512],
                                             FT.Exp, scale=0.125)
                        nc.vector.tensor_tensor(out=es[:, d0:d0 + P],
                                                in0=es[:, d0:d0 + P],
                                                in1=dmask, op=OP.mult)
                    else:
                        nc.scalar.activation(es[:], s_ps[:], FT.Exp, scale=0.125)

                for ri, ck in enumerate(range(0, 8, 2)):
                    ess = {}
                    for kt in range(ck, ck + 2):
                        for u in range(2):
                            rows = slice(64 * u, 64 * (u + 1))
                            for qb in range(2):
                                if kt >= 4 * (qb + 1):
                                    continue
                                mi = kt - 4 * qb
                                d0 = 128 * mi if 0 <= mi < 4 else 0
                                s_ps = pp.tile([P, 512], f32, tag="ps")
                                nc.tensor.matmul(
                                    s_ps[:, d0:512],
                                    kT[rows, P * kt:P * (kt + 1)],
                                    qT[rows, 512 * qb + d0:512 * (qb + 1)],
                                    start=True, stop=True)
                                es = esp.tile([P, 512], bf16, tag="es")
                                emit_exp(es, s_ps, mi)
                                ess[(kt, u, qb)] = es
                    if ri < len(groups_nxt):
                        groups_nxt[ri]()  # PE filler while this round's exps drain
                    for kt in range(ck, ck + 2):
                        for u in range(2):
                            a = 2 * p + u
                            for qb in range(2):
                                if kt >= 4 * (qb + 1):
                                    continue
                                mi = kt - 4 * qb
                                d0 = 128 * mi if 0 <= mi < 4 else 0
                                nc.tensor.matmul(
                                    pv_ps[(qb, u)][:, d0:512],
                                    v_tiles[kt][:, 65 * a:65 * a + 65],
                                    ess[(kt, u, qb)][:, d0:512],
                                    start=(kt == 0),
                                    stop=(kt == 4 * (qb + 1) - 1))
                for qb in range(2):
                    for u in range(2):
                        rrow = rp.tile([1, 512], f32, tag="rrow")
                        nc.vector.reciprocal(rrow[:], pv_ps[(qb, u)][64:65, :])
                        R = rp.tile([64, 512], f32, tag="R")
                        nc.gpsimd.partition_broadcast(R[:], rrow[:])
                        nc.vector.tensor_tensor(
                            out=attout[p][64 * u:64 * (u + 1),
                                          512 * qb:512 * (qb + 1)],
                            in0=pv_ps[(qb, u)][0:64, :], in1=R[:], op=OP.mult)
                if dsts_nxt is not None:
                    qk_cur = dsts_nxt

            # ---- phase D: proj (token-major) + residual into btc tiles ----
            for tt in range(8):
                if tt == 0 and "pss" in proj_state:
                    pss = proj_state["pss"]
                    for cb in range(2):
                        nc.tensor.matmul(pss[cb][:], attout[7][:, 0:P],
                                         prch[cb][:, 7, :],
                                         start=False, stop=True)
                else:
                    pss = [pp.tile([P, 512], f32, tag="ps",
                                   name=f"prps{tt}_{cb}") for cb in range(2)]
                    for kb in range(8):
                        for cb in range(2):
                            nc.tensor.matmul(pss[cb][:],
                                             attout[kb][:, P * tt:P * (tt + 1)],
                                             prch[cb][:, kb, :],
                                             start=(kb == 0), stop=(kb == 7))
                for cb in range(2):
                    sl = btc_tiles[tt][:, 512 * cb:512 * (cb + 1)]
                    nc.vector.tensor_tensor(out=sl, in0=pss[cb][:], in1=sl,
                                            op=OP.add)

          # ---- phase E: LN2 + transposes -> ylnT (reuses lnT tags) ----
          ylnT3 = lnTp.tile([P, 8, T], bf16, tag="xT", name="ylnT3")
          with tc.tile_pool(name="mlp", bufs=1) as mp_, \
               tc.tile_pool(name="wm2p", bufs=3) as wm2p, \
               tc.tile_pool(name="m1w", bufs=6) as m1w, \
               tc.tile_pool(name="outp", bufs=3) as outp:
            pre_wch = []
            for co in range(6):
                wch = m1w.tile([P, 8, P], bf16, tag="m1w")
                nc.sync.dma_start(wch[:], d_wm1[:, co, :, :])
                pre_wch.append(wch)
            for i in range(8):
                yn = layernorm_tile(btc_tiles[i])
                if trivial_affine:
                    nc.sync.dma_start_transpose(
                        ylnT3[:, :, P * i:P * (i + 1)], yn[:])
                else:
                    transposes_for(i, yn, ylnT3, 16, 24)

            # ---- phase F: m1 + GELU -> h tiles (SBUF-resident) ----
            h_tiles = []
            for blk in range(0, 32, 4):
                batch = []
                for co in range(blk, blk + 4):
                    if co < 6:
                        wch = pre_wch[co]
                    else:
                        wch = m1w.tile([P, 8, P], bf16, tag="m1w")
                        nc.sync.dma_start(wch[:], d_wm1[:, co, :, :])
                    ht = mp_.tile([P, T], bf16, tag=f"h{co}")
                    batch.append((co, wch, ht))
                    h_tiles.append(ht)
                # qb0 passes for the whole block first: they need only LN2
                # token tiles 0-3, so m1 starts while tiles 4-7 still
                # normalize/transpose.
                for qb in range(2):
                    for co, wch, ht in batch:
                        ps = pp.tile([P, 512], f32, tag="ps",
                                     name=f"m1ps{co}_{qb}")
                        for kb in range(8):
                            nc.tensor.matmul(
                                ps[:], wch[:, kb, :],
                                ylnT3[:, kb, 512 * qb:512 * (qb + 1)],
                                start=(kb == 0), stop=(kb == 7))
                        nc.scalar.activation(ht[:, 512 * qb:512 * (qb + 1)],
                                             ps[:], FT.Gelu,
                                             bias=cf[:, 32 + co:33 + co])

            # ---- phase G: m2 feature-major, residual seeded via transposes,
            #      32-deep PSUM accumulation, direct PSUM->DRAM output ----
            for co in range(8):
                w2 = wm2p.tile([P, 32, P], bf16, tag="w2")
                nc.sync.dma_start(w2[:], d_wm2[:, co, :, :])
                for qb in range(2):
                    ps = pp.tile([P, 512], f32, tag="ps",
                                 name=f"m2ps{co}_{qb}")
                    for l in range(4):
                        nc.tensor.matmul(
                            ps[:, P * l:P * (l + 1)],
                            btc_tiles[4 * qb + l][:, P * co:P * (co + 1)],
                            identf, is_transpose=True,
                            start=(l == 0), stop=False)
                    for kb in range(32):
                        nc.tensor.matmul(ps[:], w2[:, kb, :],
                                         h_tiles[kb][:, 512 * qb:512 * (qb + 1)],
                                         start=False, stop=(kb == 31))
                    ot = outp.tile([P, 512], f32, tag="ot")
                    nc.scalar.copy(ot[:], ps[:])
                    nc.sync.dma_start(
                        d_outT[P * co:P * (co + 1), 512 * qb:512 * (qb + 1)],
                        ot[:])

    nc.compile()
    _NC_CACHE[key] = nc
    return nc


def _prep_inputs(inputs):
    """Host-side preprocessing: LoRA fold, contiguous weight layouts, casts."""
    f = np.float32
    s = 1.0 / 16.0
    wqkv = np.asarray(inputs["qkv_w"], f) + s * (
        np.asarray(inputs["qkv_A"], f) @ np.asarray(inputs["qkv_B"], f))
    wm1 = np.asarray(inputs["m1_w"], f) + s * (
        np.asarray(inputs["m1_A"], f) @ np.asarray(inputs["m1_B"], f))
    wm2 = np.asarray(inputs["m2_w"], f) + s * (
        np.asarray(inputs["m2_A"], f) @ np.asarray(inputs["m2_B"], f))
    wpj = np.asarray(inputs["proj_w"], f)

    b8 = ml_dtypes.bfloat16
    qk = wqkv[:, 0:2048].astype(b8)     # [C, 2C] -> [p, co16, kb8, 128]
    WQK = np.ascontiguousarray(qk.reshape(8, P, 16, P).transpose(1, 2, 0, 3))
    WV = np.ascontiguousarray(
        wqkv[:, 2048:3072].astype(b8).reshape(8, P, C).transpose(1, 0, 2))
    WPJ = np.ascontiguousarray(
        wpj.astype(b8).reshape(8, P, 2, 512).transpose(1, 2, 0, 3))
    WM1 = np.ascontiguousarray(
        wm1.astype(b8).reshape(8, P, 32, P).transpose(1, 2, 0, 3))
    WM2 = np.ascontiguousarray(
        wm2.astype(b8).reshape(32, P, 8, P).transpose(1, 2, 0, 3))

    def re_pc(v, n):  # [n*128] -> [128, n] with c = 128*j + p
        return np.ascontiguousarray(np.asarray(v, f).reshape(n, P).T)

    qi = np.arange(P)[None, :]
    ki = np.arange(P)[:, None]
    dmask = (qi - ki >= 0).astype(f)

    constsf = np.concatenate([
        re_pc(inputs["ln1_g"], 8), re_pc(inputs["ln1_b"], 8),
        re_pc(inputs["ln2_g"], 8), re_pc(inputs["ln2_b"], 8),
        re_pc(inputs["m1_b"], 32), np.eye(P, dtype=f)], axis=1)
    constsb = np.concatenate([np.eye(P, dtype=f), dmask],
                             axis=1).astype(b8)
    common = {
        "wqk": WQK, "wv": WV, "wpj": WPJ, "wm1": WM1, "wm2": WM2,
        "constsf": np.ascontiguousarray(constsf),
        "constsb": np.ascontiguousarray(constsb),
    }
    btc = np.asarray(inputs["btc"], f)
    return [dict(common, btc=np.ascontiguousarray(btc[c]))
            for c in range(N_CORES)]


def _make_runner(nc):
    """Persistent sharded jit over the 8 cores (cached across kernel() calls)."""
    import jax
    from jax.sharding import Mesh, PartitionSpec
    from jax.experimental.shard_map import shard_map
    from concourse.bass2jax import (_bass_exec_p, install_neuronx_cc_hook,
                                    partition_id_tensor)
    install_neuronx_cc_hook()
    in_names, out_names, out_avals, zero_outs = [], [], [], []
    for alloc in nc.m.functions[0].allocations:
        if not isinstance(alloc, mybir.MemoryLocationSet):
            continue
        name = alloc.memorylocations[0].name
        if alloc.kind == "ExternalInput":
            in_names.append(name)
        elif alloc.kind == "ExternalOutput":
            out_names.append(name)
            shape = tuple(alloc.tensor_shape)
            dtype = mybir.dt.np(alloc.dtype)
            out_avals.append(jax.core.ShapedArray(shape, dtype))
            zero_outs.append(np.zeros(shape, dtype))
    pname = nc.partition_id_tensor.name if nc.partition_id_tensor else None
    if pname is not None and pname in in_names:
        in_names.remove(pname)
    n_params = len(in_names)
    all_in_names = in_names + out_names + ([pname] if pname else [])

    def _body(*args):
        operands = list(args)
        if pname is not None:
            operands.append(partition_id_tensor())
        outs = _bass_exec_p.bind(
            *operands, out_avals=tuple(out_avals), in_names=tuple(all_in_names),
            out_names=tuple(out_names), lowering_input_output_aliases=(),
            sim_require_finite=True, sim_require_nnan=True, nc=nc)
        return tuple(outs)

    devices = jax.devices()[:N_CORES]
    mesh = Mesh(np.asarray(devices), ("core",))
    specs = (PartitionSpec("core"),) * (n_params + len(out_names))
    fn = jax.jit(shard_map(_body, mesh=mesh, in_specs=specs,
                           out_specs=(PartitionSpec("core"),) * len(out_names),
                           check_rep=False), keep_unused=True)

    def run(in_maps):
        args = []
        for name in in_names:
            args.append(np.concatenate([np.asarray(m[name]) for m in in_maps],
                                       axis=0))
        for z in zero_outs:
            args.append(np.zeros((N_CORES * z.shape[0], *z.shape[1:]), z.dtype))
        out_arrs = fn(*args)
        return [
            {name: np.asarray(out_arrs[i]).reshape(N_CORES, *out_avals[i].shape)[c]
             for i, name in enumerate(out_names)}
            for c in range(N_CORES)]

    return run


def kernel(**inputs):
    f = np.float32
    trivial = (np.all(np.asarray(inputs["ln1_g"], f) == 1.0)
               and np.all(np.asarray(inputs["ln1_b"], f) == 0.0)
               and np.all(np.asarray(inputs["ln2_g"], f) == 1.0)
               and np.all(np.asarray(inputs["ln2_b"], f) == 0.0))
    nc = build_nc(trivial_affine=trivial)
    rkey = ("runner", trivial)
    if rkey not in _NC_CACHE:
        _NC_CACHE[rkey] = _make_runner(nc)
    in_maps = _prep_inputs(inputs)
    results = _NC_CACHE[rkey](in_maps)
    out = np.stack([results[c]["outT"].T for c in range(N_CORES)])
    out += np.asarray(inputs["m2_b"], np.float32)[None, None, :]
    return np.ascontiguousarray(out.astype(np.float32))


# revision 46
# speedup vs baseline: 1.1277x; 1.1277x over previous
"""Trainium2 Bass kernel for nn_Block_38285338477091 (dense transformer block).

Strategy:
- Data-parallel over batch: 8 NeuronCores, one batch element [1024,1024] each,
  weights replicated, zero collectives.
- LoRA folded into the base weights host-side (x@W + ((x@A)@B)*s == x@(W + s*A@B)).
- Weight-stationary transposed-activation layout for linear layers;
  LayerNorm token-major (bn_stats over free dim), PE transpose to
  feature-major with the g/b affine fused into PSUM eviction.
- LN feature-transposes go through the DMA XBAR (dma_start_transpose, 8
  per LN phase) when the LN affine is trivial (g==1/b==0, guaranteed by
  the input spec) — no PE transposes or DVE evictions; a PE-transpose
  fallback handles general affines.
- Startup pipelining: v-GEMM groups (2 token-tiles each) are emitted
  interleaved with the per-tile LN chain so the PE starts real
  contraction work while btc tiles are still streaming in from HBM.
- Causal attention: heads packed two-per-array; softmax without
  max-subtraction; per-column sums ride the PV matmul as a 65th "ones"
  column of V; masked blocks skipped (k>q) or masked post-exp.
- Attention software pipeline: head-pair p+1's q/k GEMM is split into 4
  one-PSUM-bank groups (8 matmuls + immediate eviction) interleaved as
  PE fillers into p's exp-paced score/PV rounds, sized so pv(4) +
  qk-filler(1-2) + score psums fit the 8 PSUM banks.
- h ([4096,1024] bf16) stays fully in SBUF (no DRAM spill); attention-phase
  pools are scope-released before the MLP-phase pools allocate.
- m2 computed feature-major: out^T[c,t] accumulated 32-deep in PSUM, with
  the residual btc^T seeded into PSUM via PE transposes (no DVE adds),
  and the finished PSUM tile DMA'd straight to DRAM (out^T layout;
  host transposes back and adds m2_b).
- All weight DMAs are fully contiguous per partition (host pre-arranged
  [p, chunk, ...] layouts).
- All matmuls bf16 (fp32 accumulation in PSUM); LN statistics and softmax
  normalization in fp32.
"""
import numpy as np
import ml_dtypes
from contextlib import ExitStack

import concourse.bass as bass
import concourse.tile as tile
from concourse import bacc, mybir
from concourse.bass_utils import run_bass_kernel_spmd  # noqa: F401 (fallback path)

f32 = mybir.dt.float32
bf16 = mybir.dt.bfloat16
FT = mybir.ActivationFunctionType
OP = mybir.AluOpType

P = 128
T = 1024
C = 1024
EPS = 1e-5
N_CORES = 8

_NC_CACHE = {}


def build_nc(reps=1, trivial_affine=True):
    key = ("nc", reps, trivial_affine)
    if key in _NC_CACHE:
        return _NC_CACHE[key]
    nc = bacc.Bacc("TRN2", target_bir_lowering=False, debug=False)

    d_btc = nc.dram_tensor("btc", [T, C], f32, kind="ExternalInput").ap()
    d_wqk = nc.dram_tensor("wqk", [P, 16, 8, P], bf16, kind="ExternalInput").ap()
    d_wv = nc.dram_tensor("wv", [P, 8, C], bf16, kind="ExternalInput").ap()
    d_wpj = nc.dram_tensor("wpj", [P, 2, 8, 512], bf16, kind="ExternalInput").ap()
    d_wm1 = nc.dram_tensor("wm1", [P, 32, 8, P], bf16, kind="ExternalInput").ap()
    d_wm2 = nc.dram_tensor("wm2", [P, 8, 32, P], bf16, kind="ExternalInput").ap()
    # packed constants: [g1|b1|g2|b2 (8 each) | m1b (32) | identf (128)] f32,
    # [identb | dmask] bf16 — two DMAs total at kernel start.
    d_cf = nc.dram_tensor("constsf", [P, 192], f32, kind="ExternalInput").ap()
    d_cb = nc.dram_tensor("constsb", [P, 256], bf16, kind="ExternalInput").ap()

    d_outT = nc.dram_tensor("outT", [C, T], f32, kind="ExternalOutput").ap()

    with tile.TileContext(nc) as tc, ExitStack() as ctx:
        consts = ctx.enter_context(tc.tile_pool(name="consts", bufs=2))
        btcp = ctx.enter_context(tc.tile_pool(name="btcp", bufs=1))
        xnp = ctx.enter_context(tc.tile_pool(name="xnp", bufs=4))
        lnTp = ctx.enter_context(tc.tile_pool(name="lnTp", bufs=1))
        wq = ctx.enter_context(tc.tile_pool(name="wq", bufs=6))
        stp = ctx.enter_context(tc.tile_pool(name="stp", bufs=8))
        rp = ctx.enter_context(tc.tile_pool(name="rp", bufs=4))
        pp = ctx.enter_context(tc.tile_pool(name="pp", bufs=8, space="PSUM"))
        for _rep in range(reps):

          # ---- constants (2 packed DMAs; emitted after btc0 below) ----
          cf = consts.tile([P, 192], f32, tag="cf")
          cb = consts.tile([P, 256], bf16, tag="cb")
          identf = cf[:, 64:192]
          identb = cb[:, 0:P]
          dmask = cb[:, P:256]
          epst = consts.tile([P, 1], f32, tag="epst")
          nc.vector.memset(epst[:], EPS)
          # activation-table warmup: pull the ACT table load for Sqrt into
          # the initial DMA window instead of the first LN use.
          warm = consts.tile([P, 1], f32, tag="warm")
          nc.scalar.activation(warm[:], epst[:], FT.Sqrt, bias=epst[:])

          # ---- helpers ----
          def layernorm_tile(x_tile):
              bn6 = stp.tile([P, 2, 6], f32, tag="bn6")
              nc.vector.bn_stats(bn6[:, 0, :], x_tile[:, 0:512])
              nc.vector.bn_stats(bn6[:, 1, :], x_tile[:, 512:1024])
              mv = stp.tile([P, 2], f32, tag="mv")
              nc.vector.bn_aggr(mv[:], bn6[:])
              sq = stp.tile([P, 1], f32, tag="sq")
              nc.scalar.activation(sq[:], mv[:, 1:2], FT.Sqrt, bias=epst[:])
              rstd = stp.tile([P, 1], f32, tag="rstd")
              nc.vector.reciprocal(rstd[:], sq[:])
              xn = xnp.tile([P, T], bf16, tag="xn")
              nc.gpsimd.tensor_scalar(out=xn[:], in0=x_tile[:],
                                      scalar1=mv[:, 0:1], scalar2=rstd[:],
                                      op0=OP.subtract, op1=OP.mult)
              return xn

          def transposes_for(i, xn, dst_tiles, g0, b0):
              """dst[j][:, 128i:+128] = xn[:, 128j:+128].T with g/b affine.
              g0/b0 are column bases into the packed cf constants tile."""
              for j in range(8):
                  trp = pp.tile([P, P], bf16, tag="ps", name=f"tr{i}_{j}")
                  nc.tensor.transpose(trp[:], xn[:, P * j:P * (j + 1)], identb)
                  nc.vector.tensor_scalar(
                      out=dst_tiles[:, j, P * i:P * (i + 1)], in0=trp[:],
                      scalar1=cf[:, g0 + j:g0 + j + 1],
                      scalar2=cf[:, b0 + j:b0 + j + 1],
                      op0=OP.mult, op1=OP.add)

          with tc.tile_pool(name="attn", bufs=1) as ap_, \
               tc.tile_pool(name="qkp", bufs=2) as qkp, \
               tc.tile_pool(name="esp", bufs=12) as esp:
            # ---- phase A: btc + wv DMAs interleaved; LN1 + transposes + v ----
            wvch = ap_.tile([P, 8, C], bf16, tag="wv")
            btc_tiles = []

            xlnT3 = lnTp.tile([P, 8, T], bf16, tag="xT", name="xlnT3")
            v_tiles = []
            for tt in range(8):
                vt = ap_.tile([P, 1040], bf16, tag=f"v{tt}")
                vt3 = vt[:].rearrange("p (h d) -> p h d", d=65)
                nc.vector.memset(vt3[:, :, 64:65], 1.0)
                v_tiles.append(vt)

            for i in range(8):
                bt = btcp.tile([P, C], f32, tag=f"bt{i}")
                # halves: bn_stats of the first half starts 1.4us earlier
                nc.sync.dma_start(bt[:, 0:512], d_btc[P * i:P * (i + 1), 0:512])
                nc.sync.dma_start(bt[:, 512:1024],
                                  d_btc[P * i:P * (i + 1), 512:1024])
                btc_tiles.append(bt)
                if i == 0:
                    nc.sync.dma_start(cb[:], d_cb[:])
                    nc.sync.dma_start(cf[:], d_cf[:])
                    nc.sync.dma_start(wvch[:, 0:1, :], d_wv[:, 0:1, :])
                elif i == 1:
                    nc.sync.dma_start(wvch[:, 1:2, :], d_wv[:, 1:2, :])
                    nc.sync.dma_start(wvch[:, 2:4, :], d_wv[:, 2:4, :])
                elif i == 2:
                    nc.sync.dma_start(wvch[:, 4:6, :], d_wv[:, 4:6, :])
                elif i == 3:
                    nc.sync.dma_start(wvch[:, 6:8, :], d_wv[:, 6:8, :])

            for i in range(8):
                xn = layernorm_tile(btc_tiles[i])
                if trivial_affine:
                    nc.sync.dma_start_transpose(
                        xlnT3[:, :, P * i:P * (i + 1)], xn[:])
                else:
                    transposes_for(i, xn, xlnT3, 0, 8)
                pss = {cb: pp.tile([P, 512], f32, tag="ps",
                                   name=f"vps{i}_{cb}") for cb in range(2)}
                for kb in range(8):
                    for cb in range(2):
                        nc.tensor.matmul(
                            pss[cb][:],
                            xlnT3[:, kb, P * i:P * (i + 1)],
                            wvch[:, kb, 512 * cb:512 * (cb + 1)],
                            start=(kb == 0), stop=(kb == 7))
                vt3 = v_tiles[i][:].rearrange("p (h d) -> p h d", d=65)
                for cb in range(2):
                    nc.scalar.copy(
                        vt3[:, 8 * cb:8 * (cb + 1), 0:64],
                        pss[cb][:].rearrange("p (h d) -> p h d", d=64))

            # ---- phase B/C: q,k per head-pair + attention ----
            attout = [ap_.tile([P, T], bf16, tag=f"ao{p}", name=f"attout{p}")
                      for p in range(8)]

            def qk_group_ops(p):
                """q/k GEMM for head-pair p as 4 one-bank groups (8 mms +
                immediate evict each) so they can be interleaved as PE
                fillers into the previous p's exp-paced attention rounds
                without exhausting PSUM slots."""
                dsts, groups = {}, []
                for which, co in (("q", p), ("k", 8 + p)):
                    dst = qkp.tile([P, T], bf16, tag=which, name=f"{which}T{p}")
                    wch = wq.tile([P, 8, P], bf16, tag="wqk")
                    nc.sync.dma_start(wch[:], d_wqk[:, co, :, :])
                    dsts[which] = dst
                    for qb in range(2):
                        def g(wch=wch, dst=dst, qb=qb, which=which):
                            ps = pp.tile([P, 512], f32, tag="ps",
                                         name=f"qk{p}{which}{qb}")
                            for kb in range(8):
                                nc.tensor.matmul(
                                    ps[:], wch[:, kb, :],
                                    xlnT3[:, kb, 512 * qb:512 * (qb + 1)],
                                    start=(kb == 0), stop=(kb == 7))
                            nc.vector.tensor_copy(
                                dst[:, 512 * qb:512 * (qb + 1)], ps[:])
                        groups.append(g)
                return dsts, groups

            prch = {}
            proj_state = {}

            def proj_fill_groups():
                """proj(tt=0) partial matmuls as p=7's round fillers:
                attout[0..6] are complete by then; kb=7 + eviction happen
                in phase D. PSUM: pv(4) + proj(2) + s_ps(2) = 8 banks."""
                def mk(kbs):
                    def g():
                        if "pss" not in proj_state:
                            proj_state["pss"] = [
                                pp.tile([P, 512], f32, tag="ps",
                                        name=f"prps0_{cb}") for cb in range(2)]
                        pss = proj_state["pss"]
                        for kb in kbs:
                            for cb in range(2):
                                nc.tensor.matmul(
                                    pss[cb][:], attout[kb][:, 0:P],
                                    prch[cb][:, kb, :],
                                    start=(kb == 0), stop=False)
                    return g
                return [mk([0, 1]), mk([2, 3]), mk([4, 5]), mk([6])]

            qk_cur, groups = qk_group_ops(0)
            for g in groups:
                g()
            for p in range(8):
                if p == 6:
                    for cb in range(2):
                        w = ap_.tile([P, 8, 512], bf16, tag=f"pj{cb}")
                        nc.sync.dma_start(w[:], d_wpj[:, cb, :, :])
                        prch[cb] = w
                dsts_nxt, groups_nxt = (qk_group_ops(p + 1) if p < 7
                                        else (None, proj_fill_groups()))
                qT, kT = qk_cur["q"], qk_cur["k"]
                pv_ps = {(qb, u): pp.tile([65, 512], f32, tag="ps",
                                          name=f"pv{p}_{qb}_{u}")
                         for qb in range(2) for u in range(2)}

                def emit_exp(es, s_ps, mi):
                    if 0 <= mi < 4:
                        d0 = 128 * mi
                        nc.scalar.activation(es[:, d0:512], s_ps[:, d0:512],
                                             FT.Exp, scale=0.125)
                        nc.vector.tensor_tensor(out=es[:, d0:d0 + P],
                                                in0=es[:, d0:d0 + P],
                                                in1=dmask, op=OP.mult)
                    else:
                        nc.scalar.activation(es[:], s_ps[:], FT.Exp, scale=0.125)

                for ri, ck in enumerate(range(0, 8, 2)):
                    ess = {}
                    for kt in range(ck, ck + 2):
                        for u in range(2):
                            rows = slice(64 * u, 64 * (u + 1))
                            for qb in range(2):
                                if kt >= 4 * (qb + 1):
                                    continue
                                mi = kt - 4 * qb
                                d0 = 128 * mi if 0 <= mi < 4 else 0
                                s_ps = pp.tile([P, 512], f32, tag="ps")
                                nc.tensor.matmul(
                                    s_ps[:, d0:512],
                                    kT[rows, P * kt:P * (kt + 1)],
                                    qT[rows, 512 * qb + d0:512 * (qb + 1)],
                                    start=True, stop=True)
                                es = esp.tile([P, 512], bf16, tag="es")
                                emit_exp(es, s_ps, mi)
                                ess[(kt, u, qb)] = es
                    if ri < len(groups_nxt):
                        groups_nxt[ri]()  # PE filler while this round's exps drain
                    for kt in range(ck, ck + 2):
                        for u in range(2):
                            a = 2 * p + u
                            for qb in range(2):
                                if kt >= 4 * (qb + 1):
                                    continue
                                mi = kt - 4 * qb
                                d0 = 128 * mi if 0 <= mi < 4 else 0
                                nc.tensor.matmul(
                                    pv_ps[(qb, u)][:, d0:512],
                                    v_tiles[kt][:, 65 * a:65 * a + 65],
                                    ess[(kt, u, qb)][:, d0:512],
                                    start=(kt == 0),
                                    stop=(kt == 4 * (qb + 1) - 1))
                for qb in range(2):
                    for u in range(2):
                        rrow = rp.tile([1, 512], f32, tag="rrow")
                        nc.vector.reciprocal(rrow[:], pv_ps[(qb, u)][64:65, :])
                        R = rp.tile([64, 512], f32, tag="R")
                        nc.gpsimd.partition_broadcast(R[:], rrow[:])
                        nc.vector.tensor_tensor(
                            out=attout[p][64 * u:64 * (u + 1),
                                          512 * qb:512 * (qb + 1)],
                            in0=pv_ps[(qb, u)][0:64, :], in1=R[:], op=OP.mult)
                if dsts_nxt is not None:
                    qk_cur = dsts_nxt

            # ---- phase D: proj (token-major) + residual into btc tiles ----
            for tt in range(8):
                if tt == 0 and "pss" in proj_state:
                    pss = proj_state["pss"]
                    for cb in range(2):
                        nc.tensor.matmul(pss[cb][:], attout[7][:, 0:P],
                                         prch[cb][:, 7, :],
                                         start=False, stop=True)
                else:
                    pss = [pp.tile([P, 512], f32, tag="ps",
                                   name=f"prps{tt}_{cb}") for cb in range(2)]
                    for kb in range(8):
                        for cb in range(2):
                            nc.tensor.matmul(pss[cb][:],
                                             attout[kb][:, P * tt:P * (tt + 1)],
                                             prch[cb][:, kb, :],
                                             start=(kb == 0), stop=(kb == 7))
                for cb in range(2):
                    sl = btc_tiles[tt][:, 512 * cb:512 * (cb + 1)]
                    nc.vector.tensor_tensor(out=sl, in0=pss[cb][:], in1=sl,
                                            op=OP.add)

          # ---- phase E: LN2 + transposes -> ylnT (reuses lnT tags) ----
          ylnT3 = lnTp.tile([P, 8, T], bf16, tag="xT", name="ylnT3")
          with tc.tile_pool(name="mlp", bufs=1) as mp_, \
               tc.tile_pool(name="wm2p", bufs=3) as wm2p, \
               tc.tile_pool(name="m1w", bufs=6) as m1w, \
               tc.tile_pool(name="outp", bufs=3) as outp:
            pre_wch = []
            for co in range(6):
                wch = m1w.tile([P, 8, P], bf16, tag="m1w")
                nc.sync.dma_start(wch[:], d_wm1[:, co, :, :])
                pre_wch.append(wch)
            for i in range(8):
                yn = layernorm_tile(btc_tiles[i])
                if trivial_affine:
                    nc.sync.dma_start_transpose(
                        ylnT3[:, :, P * i:P * (i + 1)], yn[:])
                else:
                    transposes_for(i, yn, ylnT3, 16, 24)

            # ---- phase F: m1 + GELU -> h tiles (SBUF-resident) ----
            h_tiles = []
            for blk in range(0, 32, 4):
                batch = []
                for co in range(blk, blk + 4):
                    if co < 6:
                        wch = pre_wch[co]
                    else:
                        wch = m1w.tile([P, 8, P], bf16, tag="m1w")
                        nc.sync.dma_start(wch[:], d_wm1[:, co, :, :])
                    ht = mp_.tile([P, T], bf16, tag=f"h{co}")
                    batch.append((co, wch, ht))
                    h_tiles.append(ht)
                # qb0 passes for the whole block first: they need only LN2
                # token tiles 0-3, so m1 starts while tiles 4-7 still
                # normalize/transpose.
                for qb in range(2):
                    for co, wch, ht in batch:
                        ps = pp.tile([P, 512], f32, tag="ps",
                                     name=f"m1ps{co}_{qb}")
                        for kb in range(8):
                            nc.tensor.matmul(
                                ps[:], wch[:, kb, :],
                                ylnT3[:, kb, 512 * qb:512 * (qb + 1)],
                                start=(kb == 0), stop=(kb == 7))
                        nc.scalar.activation(ht[:, 512 * qb:512 * (qb + 1)],
                                             ps[:], FT.Gelu,
                                             bias=cf[:, 32 + co:33 + co])

            # ---- phase G: m2 feature-major, residual seeded via transposes,
            #      32-deep PSUM accumulation, direct PSUM->DRAM output ----
            for co in range(8):
                w2 = wm2p.tile([P, 32, P], bf16, tag="w2")
                nc.sync.dma_start(w2[:], d_wm2[:, co, :, :])
                for qb in range(2):
                    ps = pp.tile([P, 512], f32, tag="ps",
                                 name=f"m2ps{co}_{qb}")
                    for l in range(4):
                        nc.tensor.matmul(
                            ps[:, P * l:P * (l + 1)],
                            btc_tiles[4 * qb + l][:, P * co:P * (co + 1)],
                            identf, is_transpose=True,
                            start=(l == 0), stop=False)
                    for kb in range(32):
                        nc.tensor.matmul(ps[:], w2[:, kb, :],
                                         h_tiles[kb][:, 512 * qb:512 * (qb + 1)],
                                         start=False, stop=(kb == 31))
                    ot = outp.tile([P, 512], f32, tag="ot")
                    nc.scalar.copy(ot[:], ps[:])
                    nc.sync.dma_start(
                        d_outT[P * co:P * (co + 1), 512 * qb:512 * (qb + 1)],
                        ot[:])

    nc.compile()
    _NC_CACHE[key] = nc
    return nc


def _prep_inputs(inputs):
    """Host-side preprocessing: LoRA fold, contiguous weight layouts, casts."""
    f = np.float32
    s = 1.0 / 16.0
    wqkv = np.asarray(inputs["qkv_w"], f) + s * (
        np.asarray(inputs["qkv_A"], f) @ np.asarray(inputs["qkv_B"], f))
    wm1 = np.asarray(inputs["m1_w"], f) + s * (
        np.asarray(inputs["m1_A"], f) @ np.asarray(inputs["m1_B"], f))
    wm2 = np.asarray(inputs["m2_w"], f) + s * (
        np.asarray(inputs["m2_A"], f) @ np.asarray(inputs["m2_B"], f))
    wpj = np.asarray(inputs["proj_w"], f)

    b8 = ml_dtypes.bfloat16
    qk = wqkv[:, 0:2048].astype(b8)     # [C, 2C] -> [p, co16, kb8, 128]
    WQK = np.ascontiguousarray(qk.reshape(8, P, 16, P).transpose(1, 2, 0, 3))
    WV = np.ascontiguousarray(
        wqkv[:, 2048:3072].astype(b8).reshape(8, P, C).transpose(1, 0, 2))
    WPJ = np.ascontiguousarray(
        wpj.astype(b8).reshape(8, P, 2, 512).transpose(1, 2, 0, 3))
    WM1 = np.ascontiguousarray(
        wm1.astype(b8).reshape(8, P, 32, P).transpose(1, 2, 0, 3))
    WM2 = np.ascontiguousarray(
        wm2.astype(b8).reshape(32, P, 8, P).transpose(1, 2, 0, 3))

    def re_pc(v, n):  # [n*128] -> [128, n] with c = 128*j + p
        return np.ascontiguousarray(np.asarray(v, f).reshape(n, P).T)

    qi = np.arange(P)[None, :]
    ki = np.arange(P)[:, None]
    dmask = (qi - ki >= 0).astype(f)

    constsf = np.concatenate([
        re_pc(inputs["ln1_g"], 8), re_pc(inputs["ln1_b"], 8),
        re_pc(inputs["ln2_g"], 8), re_pc(inputs["ln2_b"], 8),
        re_pc(inputs["m1_b"], 32), np.eye(P, dtype=f)], axis=1)
    constsb = np.concatenate([np.eye(P, dtype=f), dmask],
                             axis=1).astype(b8)
    common = {
        "wqk": WQK, "wv": WV, "wpj": WPJ, "wm1": WM1, "wm2": WM2,
        "constsf": np.ascontiguousarray(constsf),
        "constsb": np.ascontiguousarray(constsb),
    }
    btc = np.asarray(inputs["btc"], f)
    return [dict(common, btc=np.ascontiguousarray(btc[c]))
            for c in range(N_CORES)]


def _make_runner(nc):
    """Persistent sharded jit over the 8 cores (cached across kernel() calls)."""
    import jax
    from jax.sharding import Mesh, PartitionSpec
    from jax.experimental.shard_map import shard_map
    from concourse.bass2jax import (_bass_exec_p, install_neuronx_cc_hook,
                                    partition_id_tensor)
    install_neuronx_cc_hook()
    in_names, out_names, out_avals, zero_outs = [], [], [], []
    for alloc in nc.m.functions[0].allocations:
        if not isinstance(alloc, mybir.MemoryLocationSet):
            continue
        name = alloc.memorylocations[0].name
        if alloc.kind == "ExternalInput":
            in_names.append(name)
        elif alloc.kind == "ExternalOutput":
            out_names.append(name)
            shape = tuple(alloc.tensor_shape)
            dtype = mybir.dt.np(alloc.dtype)
            out_avals.append(jax.core.ShapedArray(shape, dtype))
            zero_outs.append(np.zeros(shape, dtype))
    pname = nc.partition_id_tensor.name if nc.partition_id_tensor else None
    if pname is not None and pname in in_names:
        in_names.remove(pname)
    n_params = len(in_names)
    all_in_names = in_names + out_names + ([pname] if pname else [])

    def _body(*args):
        operands = list(args)
        if pname is not None:
            operands.append(partition_id_tensor())
        outs = _bass_exec_p.bind(
            *operands, out_avals=tuple(out_avals), in_names=tuple(all_in_names),
            out_names=tuple(out_names), lowering_input_output_aliases=(),
            sim_require_finite=True, sim_require_nnan=True, nc=nc)
        return tuple(outs)

    devices = jax.devices()[:N_CORES]
    mesh = Mesh(np.asarray(devices), ("core",))
    specs = (PartitionSpec("core"),) * (n_params + len(out_names))
    fn = jax.jit(shard_map(_body, mesh=mesh, in_specs=specs,
                           out_specs=(PartitionSpec("core"),) * len(out_names),
                           check_rep=False), keep_unused=True)

    def run(in_maps):
        args = []
        for name in in_names:
            args.append(np.concatenate([np.asarray(m[name]) for m in in_maps],
                                       axis=0))
        for z in zero_outs:
            args.append(np.zeros((N_CORES * z.shape[0], *z.shape[1:]), z.dtype))
        out_arrs = fn(*args)
        return [
            {name: np.asarray(out_arrs[i]).reshape(N_CORES, *out_avals[i].shape)[c]
             for i, name in enumerate(out_names)}
            for c in range(N_CORES)]

    return run


def kernel(**inputs):
    f = np.float32
    trivial = (np.all(np.asarray(inputs["ln1_g"], f) == 1.0)
               and np.all(np.asarray(inputs["ln1_b"], f) == 0.0)
               and np.all(np.asarray(inputs["ln2_g"], f) == 1.0)
               and np.all(np.asarray(inputs["ln2_b"], f) == 0.0))
    nc = build_nc(trivial_affine=trivial)
    rkey = ("runner", trivial)
    if rkey not in _NC_CACHE:
        _NC_CACHE[rkey] = _make_runner(nc)
    in_maps = _prep_inputs(inputs)
    results = _NC_CACHE[rkey](in_maps)
    out = np.stack([results[c]["outT"].T for c in range(N_CORES)])
    out += np.asarray(inputs["m2_b"], np.float32)[None, None, :]
    return np.ascontiguousarray(out.astype(np.float32))
